# revision 29
# baseline (speedup 1.0000x reference)
"""Fused GroupNorm + legacy-split multi-head attention + 1x1 projection with
residual, for x:(2, 256, 64, 64), on 8 Trainium2 NeuronCores.

Sharding: core i = 4*b + j handles batch b and t-slice j (1024 of the 4096
flattened spatial positions). k/v are computed for the full sequence on every
core of a batch group (cheap, redundant); each core's projection output slice
is complete, so the host only concatenates slices — no collectives.

SPMD: all cores run the identical program. The host rotates each core's copy
of x along t so the core's own slice sits at columns 0:1024 (GroupNorm stats
and the attention contraction over s are invariant to a consistent
permutation of the contracted axis).

Math layout notes:
- scores are computed transposed, S^T[s, t] = k^T q, so softmax's reduction
  runs over the PSUM partition dim; the denominator comes for free from a
  ones-column appended to v^T in the a = w v matmul (output row 64).
- no max-subtraction in softmax: scores are ~N(0, 1) with |s| < ~8, exp is
  safe in fp32 (verified against the reference on host).
- exp is split across engines: heads 2p (hh=0) use the ACT spline exp;
  heads 2p+1 (hh=1) use a Schraudolph fast-exp on the DVE — w_bits_i16 =
  round(s * A + B) bitcast to f16 (max ~3% weight error, ~2e-4 on the
  final output after softmax normalization and head mixing).
- q/k biases are added during the PSUM->SBUF copies; the v bias is folded
  into the projection bias on the host; the attention scale 1/sqrt(ch) is
  folded into exp's scale argument.
- matmuls run in float32r (~1.2e-4 relative rounding, full PE speed).

Performance notes (measured on HW, paired A/B):
- The kernel is power/clock limited: sustained full-rate PE across all 8
  cores trips a clock clamp (K=4/8, half PE clock) ~140us in. The zeros
  "dummy" matmuls into the open av accumulation are load-bearing: they keep
  the HAM activity window busy at low switching power, and the clamp then
  releases in ~17us instead of persisting ~140us (paired cost of removing
  them: +50us). 2x512 cols per sparse j is the measured optimum (1x: +17us,
  3x: +3us).
- GroupNorm rstd is computed on the DVE (bit-hack seed + 2 Newton steps) so
  ACT loads its exp table exactly once, at t~0 - an ACT_TABLE_LOAD swap
  mid-kernel costs 2.7us on the critical path.
- The k-bias PSUM->SBUF copy runs on ACT but is emitted 2 side-slots after
  its matmuls: ACT's 8-deep FIFO otherwise head-of-line-blocks the next exp
  behind an unfinished k matmul.
- Per j, both heads' S matmuls+exps are emitted before both AV accumulations
  so one exp engine's stall cannot cascade into the other head's chain
  through the PE FIFO.
- x DMA: slices first on all three queues (tiny-const DMAs ahead of x cost
  1-2us completion latency each); PE warm-up matmuls are paced by the
  landing x slices (f16 bitcast views) to bridge HAM until the stream.
- Keeping k/v production interleaved with pair-0 attention beats a dedicated
  production phase by ~17us: production rides in the exp-chain stall shadow.
- 512-wide exp halves (to shorten the exp->S bank chain) lose ~22us: the
  extra per-op overhead on ACT/DVE outweighs the chain relief.
"""
import math
from contextlib import ExitStack

import numpy as np

import concourse.bacc as bacc
import concourse.tile as tile
from concourse import mybir
from concourse.bass_utils import run_bass_kernel_spmd

f32 = mybir.dt.float32
f32r = mybir.dt.float32r
f16 = mybir.dt.float16
i16 = mybir.dt.int16
i32 = mybir.dt.int32
FT = mybir.ActivationFunctionType
ALU = mybir.AluOpType

B, C, HH, WW = 2, 256, 64, 64
T = HH * WW           # 4096
TS = T // 4           # 1024 t-columns per core
HEADS = 4
CH = C // HEADS       # 64 channels per head
NT = TS // 512        # 512-wide matmul output tiles per t-slice
SJ = T // 128         # 32 s-tiles
EPS = 1e-5
N_CORES = 8
EXP_SCALE = 1.0 / math.sqrt(CH)  # (1/ch^0.25)^2 folded into exp
# Schraudolph fast exp on the DVE: f16 bits = round(s * SCH_A + SCH_B),
# bitcast int16 -> f16. Shift 45 minimizes max relative error (~3.0%).
SCH_A = EXP_SCALE * (1 << 10) / math.log(2.0)
SCH_B = float(15 * (1 << 10) - 45)

_CACHE: dict = {}


def _build():
    nc = bacc.Bacc("TRN2", target_bir_lowering=False, debug=False,
                   num_devices=N_CORES)

    def dram_in(name, shape, dtype=f32):
        return nc.dram_tensor(name, shape, dtype, kind="ExternalInput").ap()

    x = dram_in("x", [C, T])
    qwt = dram_in("qwt", [C, C], f16)
    kwt = dram_in("kwt", [C, C], f16)
    vwt = dram_in("vwt", [C, C], f16)
    pwt = dram_in("pwt", [C, C], f16)
    qb2 = dram_in("qb2", [128, 2])
    kb2 = dram_in("kb2", [128, 2])
    pb2 = dram_in("pb2", [128, 2])
    nw2 = dram_in("nw2", [128, 2])
    nb2 = dram_in("nb2", [128, 2])
    gsel = dram_in("gsel", [128, 16], f32r)
    gselt = dram_in("gselt", [16, 128], f32r)
    ones = dram_in("ones", [128, 128], f16)
    out = nc.dram_tensor("out", [C, TS], f32, kind="ExternalOutput").ap()

    x2 = x.rearrange("(i p) t -> p i t", i=2)  # [128, 2, 4096] view

    with tile.TileContext(nc) as tc, ExitStack() as ctx:
        sb1 = ctx.enter_context(tc.tile_pool(name="sb1", bufs=1))
        wp = ctx.enter_context(tc.tile_pool(name="wp", bufs=4))
        st = ctx.enter_context(tc.tile_pool(name="st", bufs=2))
        rp = ctx.enter_context(tc.tile_pool(name="rp", bufs=2))
        ps = ctx.enter_context(tc.tile_pool(name="ps", bufs=1, space="PSUM"))
        psa = ctx.enter_context(tc.tile_pool(name="psa", bufs=1, space="PSUM"))

        # ---- persistent tiles ----
        # DMA queue plan (3 queues: SP + ACT hardware DGE ~150 GB/s each,
        # gpsimd software DGE ~86 GB/s). Tiny constants go first, then the
        # 8 x-slices balanced 3/3/2, large weights after x on the queues.
        x_sb = sb1.tile([128, 2, T], f32)
        qb_sb = sb1.tile([128, 2], f32)
        kb_sb = sb1.tile([128, 2], f32)
        pb_sb = sb1.tile([128, 2], f32)
        nw_sb = sb1.tile([128, 2], f32)
        nb_sb = sb1.tile([128, 2], f32)
        gsel_sb = sb1.tile([128, 16], f32r)
        gselt_sb = sb1.tile([16, 128], f32r)
        ones_sb = sb1.tile([128, 128], f16)
        qwt_sb = sb1.tile([128, 2, C], f16)
        kwt_sb = sb1.tile([128, 2, C], f16)
        vwt_sb = sb1.tile([128, 2, C], f16)
        pwt_sb = sb1.tile([128, 2, C], f16)
        eps_sb = sb1.tile([128, 1], f32)
        nc.vector.memset(eps_sb[:], EPS)

        def xdma(eng, i, c2):
            eng.dma_start(out=x_sb[:, i, c2 * 1024:(c2 + 1) * 1024],
                          in_=x2[:, i, c2 * 1024:(c2 + 1) * 1024])

        def wdma(eng, dst, src):
            eng.dma_start(out=dst[:], in_=src.rearrange("(i p) o -> p i o", i=2))

        # x slices first on every queue (tiny-const DMAs ahead of x cost
        # ~1-2us completion latency each); constants and weights after.
        # sync queue: 3 x-slices, group-norm constants, k/v weights
        xdma(nc.sync, 0, 0)
        xdma(nc.sync, 0, 1)
        xdma(nc.sync, 0, 2)
        for dst, src in ((gsel_sb, gsel), (gselt_sb, gselt), (nw_sb, nw2),
                         (nb_sb, nb2), (qb_sb, qb2), (kb_sb, kb2)):
            nc.sync.dma_start(out=dst[:], in_=src[:])
        wdma(nc.sync, kwt_sb, kwt)
        wdma(nc.sync, vwt_sb, vwt)
        # scalar queue: 3 x-slices, then q weights + tail bias
        xdma(nc.scalar, 0, 3)
        xdma(nc.scalar, 1, 0)
        xdma(nc.scalar, 1, 1)
        wdma(nc.scalar, qwt_sb, qwt)
        nc.scalar.dma_start(out=pb_sb[:], in_=pb2[:])
        # gpsimd queue (slow SWDGE): ones (needed mid-head-phase), 2 x-slices
        nc.gpsimd.dma_start(out=ones_sb[:], in_=ones[:])
        xdma(nc.gpsimd, 1, 2)
        xdma(nc.gpsimd, 1, 3)
        wdma(nc.gpsimd, pwt_sb, pwt)

        xn = sb1.tile([128, 2, T], f16)
        k_sb = sb1.tile([128, 2, T], f16)
        q_sb = sb1.tile([128, 2, TS], f16)
        vaug = sb1.tile([128, SJ, HEADS, CH + 1], f16)
        a_sb = sb1.tile([128, 2, TS], f16)

        # Load the natural_log_exp activation table once, at t~0 (GroupNorm
        # rstd uses Ln+Exp and the attention stream uses Exp, so ACT never
        # swaps table sets mid-kernel).
        exp_warm = st.tile([16, 1], f32, name="exp_warm", tag="expw")
        nc.scalar.activation(out=exp_warm[:], in_=eps_sb[0:16, :], func=FT.Exp)

        # PE warm-up: an initial burst on a memset tile ramps HAM to 8/8,
        # then matmuls paced by the landing x slices (f16 bitcast views of
        # the f32 data - values are garbage, the DMA dependency is the
        # point) keep every HAM activity window non-idle until the real
        # matmul stream starts.
        warm16 = sb1.tile([128, 512], f16)
        nc.vector.memset(warm16[:], 1.0)
        warm_ps = ps.tile([128, 512], f32, name="warm_ps", tag="sc0")
        for _ in range(20):
            nc.tensor.matmul(out=warm_ps[:], lhsT=warm16[:, 0:128],
                             rhs=warm16[:], start=True, stop=True)
        warm_order = ((0, 0), (0, 3), (1, 2), (0, 1), (1, 0), (0, 2), (1, 1))
        for n, (i, c2) in enumerate(warm_order):
            reps = 5 if n < 5 else 3
            for r in range(reps):
                base = c2 * 1024 + r * 192
                nc.tensor.matmul(
                    out=warm_ps[:],
                    lhsT=warm16[:, 0:128],
                    rhs=x_sb[:, i, base:base + 256].bitcast(f16),
                    start=True, stop=True,
                )

        # zeros tile for the HAM-keepalive dummies in the attention stream
        zer_sb = sb1.tile([128, 65], f16)
        nc.vector.memset(zer_sb[:], 0.0)

        # ones column of vaug (col CH of every (j, h) slot) - on gpsimd so
        # the wait for the ones DMA doesn't head-of-line-block the DVE
        nc.gpsimd.tensor_copy(
            out=vaug[:, :, :, CH:CH + 1],
            in_=ones_sb[:, 0:SJ * HEADS].rearrange("p (j h) -> p j h", j=SJ),
        )

        # ---- phase A: GroupNorm ----
        stats_all = sb1.tile([128, 2, 8, 6], f32)
        ab = []  # per c-tile (alpha, beta) [128, 2]
        for i in range(2):
            for s in range(8):
                nc.vector.bn_stats(
                    out=stats_all[:, i, s, :],
                    in_=x_sb[:, i, s * 512:(s + 1) * 512],
                )
            hp = tc.high_priority()
            hp.__enter__()
            mv = st.tile([128, 2], f32, name=f"mv_{i}", tag="mv")
            nc.vector.bn_aggr(out=mv[:], in_=stats_all[:, i])
            # me = (mean_c, E[x^2]_c)
            me = st.tile([128, 2], f32, name=f"me_{i}", tag="me")
            nc.vector.tensor_copy(out=me[:, 0:1], in_=mv[:, 0:1])
            nc.vector.tensor_tensor(out=me[:, 1:2], in0=mv[:, 0:1], in1=mv[:, 0:1], op=ALU.mult)
            nc.vector.tensor_add(out=me[:, 1:2], in0=me[:, 1:2], in1=mv[:, 1:2])
            me_r = st.tile([128, 2], f32r, name=f"me_r_{i}", tag="me_r")
            nc.vector.tensor_copy(out=me_r[:], in_=me[:])
            # group sums: [16, 2] = sum over the 8 channels of each group
            gs_ps = ps.tile([16, 2], f32, name=f"gs_ps_{i}", tag="sc0")
            nc.tensor.matmul(out=gs_ps[:], lhsT=gsel_sb[:], rhs=me_r[:], start=True, stop=True)
            gstats = st.tile([16, 2], f32, name=f"gstats_{i}", tag="gstats")
            nc.vector.tensor_scalar_mul(out=gstats[:], in0=gs_ps[:], scalar1=1.0 / 8.0)
            tmp1 = st.tile([16, 1], f32, name=f"tmp1_{i}", tag="tmp1")
            nc.vector.tensor_tensor(out=tmp1[:], in0=gstats[:, 0:1], in1=gstats[:, 0:1], op=ALU.mult)
            nc.vector.tensor_sub(out=gstats[:, 1:2], in0=gstats[:, 1:2], in1=tmp1[:])
            # rstd = 1/sqrt(var + eps) entirely on the DVE (bit-hack seed +
            # two Newton steps) so ACT keeps its exp table resident for the
            # whole kernel - no ACT_TABLE_LOAD swaps.
            v_t = st.tile([16, 1], f32, name=f"v_{i}", tag="rsq_v")
            nc.vector.tensor_scalar_add(out=v_t[:], in0=gstats[:, 1:2],
                                        scalar1=eps_sb[0:16, :])
            hsh = st.tile([16, 1], f32, name=f"h_{i}", tag="rsq_h")
            nc.vector.tensor_scalar_mul(out=hsh[:], in0=v_t[:], scalar1=0.5)
            # seed bits = 0x5F3759DF - bits(v)/2, computed in fp32 (the int32
            # input converts to fp32 in the ALU; <=0.5-bit error is nothing
            # against the 3.4% seed error), then rounded back to int32
            u_t = st.tile([16, 1], f32, name=f"u_{i}", tag="rsq_u")
            nc.vector.tensor_scalar(out=u_t[:], in0=v_t[:].bitcast(i32),
                                    scalar1=-0.5, scalar2=float(0x5F3759DF),
                                    op0=ALU.mult, op1=ALU.add)
            y_t = st.tile([16, 1], f32, name=f"y_{i}", tag="rsq_y")
            nc.vector.tensor_copy(out=y_t[:].bitcast(i32), in_=u_t[:])
            t_t = st.tile([16, 1], f32, name=f"t_{i}", tag="rsq_t")
            for _ in range(2):  # Newton: y *= 1.5 - h*y*y
                nc.vector.tensor_tensor(out=t_t[:], in0=y_t[:], in1=y_t[:], op=ALU.mult)
                nc.vector.tensor_tensor(out=t_t[:], in0=t_t[:], in1=hsh[:], op=ALU.mult)
                nc.vector.tensor_scalar(out=t_t[:], in0=t_t[:], scalar1=-1.0,
                                        scalar2=1.5, op0=ALU.mult, op1=ALU.add)
                nc.vector.tensor_tensor(out=y_t[:], in0=y_t[:], in1=t_t[:], op=ALU.mult)
            nc.vector.tensor_copy(out=gstats[:, 1:2], in_=y_t[:])
            gstats_r = st.tile([16, 2], f32r, name=f"gstats_r_{i}", tag="gstats_r")
            nc.vector.tensor_copy(out=gstats_r[:], in_=gstats[:])
            # broadcast to channels: [128, 2] = (mean_c, rstd_c)
            ch_ps = ps.tile([128, 2], f32, name=f"ch_ps_{i}", tag="sc1")
            nc.tensor.matmul(out=ch_ps[:], lhsT=gselt_sb[:], rhs=gstats_r[:], start=True, stop=True)
            ab_i = st.tile([128, 2], f32, name=f"ab_{i}", tag="ab", bufs=2)
            nc.vector.tensor_tensor(out=ab_i[:, 0:1], in0=ch_ps[:, 1:2], in1=nw_sb[:, i:i + 1], op=ALU.mult)
            tmp2 = st.tile([128, 1], f32, name=f"tmp2_{i}", tag="tmp2")
            nc.vector.tensor_tensor(out=tmp2[:], in0=ch_ps[:, 0:1], in1=ab_i[:, 0:1], op=ALU.mult)
            nc.vector.tensor_sub(out=ab_i[:, 1:2], in0=nb_sb[:, i:i + 1], in1=tmp2[:])
            hp.__exit__(None, None, None)
            ab.append(ab_i)

        # apply affine -> xn (f16) in 1024-col chunks, alternating DVE/ACT to
        # halve the post-stats latency
        for i in range(2):
            for c4 in range(4):
                sl = slice(c4 * 1024, (c4 + 1) * 1024)
                if c4 % 2 == 0:
                    nc.vector.tensor_scalar(
                        out=xn[:, i, sl], in0=x_sb[:, i, sl],
                        scalar1=ab[i][:, 0:1], scalar2=ab[i][:, 1:2],
                        op0=ALU.mult, op1=ALU.add,
                    )
                else:
                    nc.scalar.activation(
                        out=xn[:, i, sl], in_=x_sb[:, i, sl],
                        func=FT.Identity,
                        scale=ab[i][:, 0:1], bias=ab[i][:, 1:2],
                    )

        # ---- phase B: qkv projections ----
        # last HAM-keepalive matmuls, paced on the last x slice
        for r in range(3):
            nc.tensor.matmul(
                out=warm_ps[:],
                lhsT=warm16[:, 0:128],
                rhs=x_sb[:, 1, 3 * 1024 + r * 192:3 * 1024 + r * 192 + 256].bitcast(f16),
                start=True, stop=True,
            )
        # q: [128, 2(pair), 1024]
        for p in range(2):
            q_ps = ps.tile([128, TS], f32, name=f"q_ps_{p}", tag=f"sc{p}")
            for nt in range(NT):
                for i in range(2):
                    nc.tensor.matmul(
                        out=q_ps[:, nt * 512:(nt + 1) * 512],
                        lhsT=qwt_sb[:, i, p * 128:(p + 1) * 128],
                        rhs=xn[:, i, nt * 512:(nt + 1) * 512],
                        start=(i == 0), stop=(i == 1),
                    )
            nc.vector.tensor_scalar_add(out=q_sb[:, p, :], in0=q_ps[:], scalar1=qb_sb[:, p:p + 1])
        # k and v^T production interleaved with attention consumption:
        # after chunk c4's k/v^T are emitted, attention js of chunk c4-1 for
        # pair 0 run, keeping ACT (exp) continuously busy from ~40us on.
        def k_thunks(c4):
            """Per pair: nt=0 matmuls, nt=1 matmuls; the PSUM->SBUF bias adds
            trail both pairs so the ACT op never head-of-line-blocks an exp
            behind an unfinished k matmul."""
            units = []
            cells = [{}, {}]
            for p in range(2):
                cell = cells[p]
                def mk_k0(p=p, cell=cell):
                    cell["t"] = ps.tile([128, 1024], f32, name=f"k_ps_{p}_{c4}", tag=f"sc{p}")
                    for i in range(2):
                        nc.tensor.matmul(
                            out=cell["t"][:, 0:512],
                            lhsT=kwt_sb[:, i, p * 128:(p + 1) * 128],
                            rhs=xn[:, i, c4 * 1024: c4 * 1024 + 512],
                            start=(i == 0), stop=(i == 1),
                        )
                def mk_k1(p=p, cell=cell):
                    for i in range(2):
                        nc.tensor.matmul(
                            out=cell["t"][:, 512:1024],
                            lhsT=kwt_sb[:, i, p * 128:(p + 1) * 128],
                            rhs=xn[:, i, c4 * 1024 + 512: c4 * 1024 + 1024],
                            start=(i == 0), stop=(i == 1),
                        )
                units += [mk_k0, mk_k1]
            for p in range(2):
                def mk_kb(p=p, cell=cells[p]):
                    # bias add on ACT (reads PSUM at full rate; keeps the DVE
                    # free for the Schraudolph exp of the hh=1 heads)
                    nc.scalar.activation(
                        out=k_sb[:, p, c4 * 1024:(c4 + 1) * 1024], in_=cell["t"][:],
                        func=FT.Identity, bias=kb_sb[:, p:p + 1],
                    )
                units.append(mk_kb)
            return units

        def v_thunk(j):
            def mk_v(j=j):
                vt_ps = ps.tile([128, C], f32, name=f"vt_ps_{j}", tag=f"sc{j % 2}")
                for i in range(2):
                    nc.tensor.matmul(
                        out=vt_ps[:], lhsT=xn[:, i, j * 128:(j + 1) * 128],
                        rhs=vwt_sb[:, i, :], start=(i == 0), stop=(i == 1),
                    )
                nc.vector.tensor_copy(
                    out=vaug[:, j, :, 0:CH],
                    in_=vt_ps.rearrange("p (h c) -> p h c", h=HEADS),
                )
            return mk_v

        att = {}  # per-pair attention state: (avs, prev_w)
        att = {}  # per-pair attention state: (avs, prev_w)

        def emit_att(p, js, side=None):
            avs, prev_w = att[p]
            side = list(side or [])
            si = 0
            per_j = max(1, (len(side) + len(js) - 1) // len(js)) if side else 0
            for j in js:
                cur_w = [None, None]
                # both heads' score matmuls + exps first (so a stall in one
                # head's exp chain can't block the other via the PE FIFO),
                # then both heads' AV accumulations (their w is ready).
                for hh in range(2):
                    h = 2 * p + hh
                    off = hh * CH
                    s_ps = ps.tile([128, TS], f32, name=f"s_ps_{h}_{j}", tag=f"sc{hh}")
                    for nt in range(NT):
                        nc.tensor.matmul(
                            out=s_ps[:, nt * 512:(nt + 1) * 512],
                            lhsT=k_sb[off:off + CH, p, j * 128:(j + 1) * 128],
                            rhs=q_sb[off:off + CH, p, nt * 512:(nt + 1) * 512],
                            start=True, stop=True,
                        )
                    w_t = wp.tile([128, TS], f16, name=f"w_{h}_{j}", tag="w")
                    cur_w[hh] = w_t
                    if hh == 0:
                        nc.scalar.activation(out=w_t[:], in_=s_ps[:], func=FT.Exp,
                                             scale=EXP_SCALE)
                    else:
                        nc.vector.tensor_scalar(
                            out=w_t[:].bitcast(i16), in0=s_ps[:],
                            scalar1=SCH_A, scalar2=SCH_B,
                            op0=ALU.mult, op1=ALU.add,
                        )
                for hh in range(2):
                    h = 2 * p + hh
                    if prev_w[hh] is not None:
                        for nt in range(NT):
                            nc.tensor.matmul(
                                out=avs[hh][:, nt * 512:(nt + 1) * 512],
                                lhsT=vaug[:, j - 1, h, :],
                                rhs=prev_w[hh][:, nt * 512:(nt + 1) * 512],
                                start=(j - 1 == 0), stop=False,
                            )
                if side and si < len(side):
                    for t in side[si:si + per_j]:
                        t()
                    si += per_j
                elif 2 <= j < SJ - 1:
                    # HAM-keepalive matmuls (add zeros to the open av
                    # accumulation) so no activity window goes idle
                    for f in range(2):
                        nc.tensor.matmul(
                            out=avs[0][:, (f % 2) * 512:(f % 2 + 1) * 512],
                            lhsT=zer_sb[:], rhs=xn[:, 0, 0:512],
                            start=False, stop=False, skip_group_check=True,
                        )
                prev_w = cur_w
            for t in side[si:]:
                t()
            att[p] = (avs, prev_w)

        def finish_stop(p):
            """Final av accumulation + evacuate av banks (raw copies)."""
            avs, prev_w = att[p]
            raws = []
            for hh in range(2):
                h = 2 * p + hh
                for nt in range(NT):
                    nc.tensor.matmul(
                        out=avs[hh][:, nt * 512:(nt + 1) * 512],
                        lhsT=vaug[:, SJ - 1, h, :],
                        rhs=prev_w[hh][:, nt * 512:(nt + 1) * 512],
                        start=False, stop=True,
                    )
            for hh in range(2):
                h = 2 * p + hh
                av = avs[hh]
                araw = rp.tile([CH, TS], f32, name=f"araw_{h}", tag=f"araw{hh}")
                nc.vector.tensor_copy(out=araw[:], in_=av[0:CH, :])
                d16 = rp.tile([1, TS], f16, name=f"d16_{h}", tag="d16")
                nc.vector.tensor_copy(out=d16[:], in_=av[CH:CH + 1, :])
                raws.append((araw, d16))
            return raws

        def normalize_thunks(p, raws):
            """Per-head normalize, emitted as side work inside the next pair."""
            thunks = []
            for hh in range(2):
                h = 2 * p + hh
                off = hh * CH
                araw, d16 = raws[hh]
                def mk(h=h, off=off, araw=araw, d16=d16, hh=hh, p=p):
                    drep_ps = ps.tile([CH, TS], f32, name=f"drep_ps_{h}", tag=f"sc{hh}")
                    for nt in range(NT):
                        nc.tensor.matmul(
                            out=drep_ps[:, nt * 512:(nt + 1) * 512],
                            lhsT=ones_sb[0:1, 0:CH],
                            rhs=d16[:, nt * 512:(nt + 1) * 512],
                            start=True, stop=True,
                        )
                    rrep_sb = rp.tile([CH, TS], f32, name=f"rrep_sb_{h}", tag="rrep")
                    nc.vector.reciprocal_approx_fast(out=rrep_sb[:], in_=drep_ps[:])
                    nc.vector.tensor_tensor(
                        out=a_sb[off:off + CH, p, :], in0=araw[:], in1=rrep_sb[:],
                        op=ALU.mult,
                    )
                thunks.append(mk)
            return thunks

        att[0] = ([psa.tile([CH + 1, TS], f32, name=f"av_{hh}", tag=f"acc{hh}")
                   for hh in range(2)], [None, None])
        for t in k_thunks(0):
            t()
        for j in range(4):
            v_thunk(j)()
        for c4 in range(4):
            side = []
            if c4 < 3:
                side += k_thunks(c4 + 1)
            side += [v_thunk(j) for j in range(8 * c4 + 4, min(8 * c4 + 12, SJ))]
            emit_att(0, range(8 * c4, 8 * c4 + 8), side=side)
        raws0 = finish_stop(0)
        att[1] = ([psa.tile([CH + 1, TS], f32, name=f"av_{2 + hh}", tag=f"acc{hh}")
                   for hh in range(2)], [None, None])
        emit_att(1, range(SJ), side=normalize_thunks(0, raws0))
        raws1 = finish_stop(1)
        for t in normalize_thunks(1, raws1):
            t()

        # ---- phase D: projection + residual ----
        for m in range(2):
            h_ps = ps.tile([128, TS], f32, name=f"h_ps_{m}", tag=f"sc{m}")
            for nt in range(NT):
                for i in range(2):
                    nc.tensor.matmul(
                        out=h_ps[:, nt * 512:(nt + 1) * 512],
                        lhsT=pwt_sb[:, i, m * 128:(m + 1) * 128],
                        rhs=a_sb[:, i, nt * 512:(nt + 1) * 512],
                        start=(i == 0), stop=(i == 1),
                    )
            o_sb = wp.tile([128, TS], f32, name=f"o_sb_{m}", tag="w")
            # bias+residual and the output DMA in 512-col halves on separate
            # queues: the first half's DMA overlaps the second half's compute
            # and the final DMA is half as long after the last compute op
            engs = ((nc.sync, nc.scalar), (nc.gpsimd, nc.sync))[m]
            for half in range(2):
                sl = slice(half * 512, (half + 1) * 512)
                nc.vector.scalar_tensor_tensor(
                    out=o_sb[:, sl], in0=h_ps[:, sl], scalar=pb_sb[:, m:m + 1],
                    in1=x_sb[:, m, sl], op0=ALU.add, op1=ALU.add,
                )
                engs[half].dma_start(
                    out=out[m * 128:(m + 1) * 128, sl], in_=o_sb[:, sl])

    nc.compile()
    return nc


def _host_inputs(x, norm_w, norm_b, qkv_w, qkv_b, proj_w, proj_b):
    """Build the 8 per-core input maps (all float32 numpy)."""
    x = np.ascontiguousarray(np.asarray(x, dtype=np.float32)).reshape(B, C, T)
    norm_w = np.asarray(norm_w, dtype=np.float32)
    norm_b = np.asarray(norm_b, dtype=np.float32)
    qkv_w = np.asarray(qkv_w, dtype=np.float32)
    qkv_b = np.asarray(qkv_b, dtype=np.float32)
    proj_w = np.asarray(proj_w, dtype=np.float32)
    proj_b = np.asarray(proj_b, dtype=np.float32)

    # head-major row gathers of the qkv conv
    q_rows = np.concatenate([np.arange(192 * h, 192 * h + 64) for h in range(HEADS)])
    k_rows = q_rows + 64
    v_rows = q_rows + 128
    qwt = np.ascontiguousarray(qkv_w[q_rows].T.astype(np.float16))
    kwt = np.ascontiguousarray(qkv_w[k_rows].T.astype(np.float16))
    vwt = np.ascontiguousarray(qkv_w[v_rows].T.astype(np.float16))
    pwt = np.ascontiguousarray(proj_w.T.astype(np.float16))

    def as2(v):  # (256,) -> [128, 2] with column p = channels 128p..128p+128
        return np.ascontiguousarray(v.reshape(2, 128).T)

    qb2 = as2(qkv_b[q_rows])
    kb2 = as2(qkv_b[k_rows])
    # v bias folded into projection bias (a_norm lacks +vb; h += proj_w @ vb)
    vb_nat = qkv_b[v_rows]  # natural channel order == head-major for v
    pb2 = as2(proj_b + proj_w @ vb_nat)
    nw2 = as2(norm_w)
    nb2 = as2(norm_b)

    gsel = np.zeros((128, 16), np.float32)
    gsel[np.arange(128), np.arange(128) // 8] = 1.0
    gselt = np.ascontiguousarray(gsel.T)
    ones = np.ones((128, 128), np.float16)

    shared = dict(qwt=qwt, kwt=kwt, vwt=vwt, pwt=pwt, qb2=qb2, kb2=kb2,
                  pb2=pb2, nw2=nw2, nb2=nb2, gsel=gsel, gselt=gselt, ones=ones)
    in_maps = []
    for core in range(N_CORES):
        b, j = core // 4, core % 4
        xr = np.concatenate([x[b][:, j * TS:], x[b][:, :j * TS]], axis=1)
        in_maps.append({"x": np.ascontiguousarray(xr), **shared})
    return in_maps


def _run(in_maps, **kw):
    if "nc" not in _CACHE:
        _CACHE["nc"] = _build()
    return run_bass_kernel_spmd(_CACHE["nc"], in_maps, list(range(N_CORES)), **kw)


def kernel(x, norm_w, norm_b, qkv_w, qkv_b, proj_w, proj_b):
    in_maps = _host_inputs(x, norm_w, norm_b, qkv_w, qkv_b, proj_w, proj_b)
    res = _run(in_maps)
    out = np.empty((B, C, T), np.float32)
    for core in range(N_CORES):
        b, j = core // 4, core % 4
        out[b][:, j * TS:(j + 1) * TS] = res.results[core]["out"]
    return out.reshape(B, C, HH, WW)



# revision 30
# speedup vs baseline: 1.1827x; 1.1827x over previous
"""Fused GroupNorm + legacy-split multi-head attention + 1x1 projection with
residual, for x:(2, 256, 64, 64), on 8 Trainium2 NeuronCores.

Sharding: core i = 4*b + j handles batch b and t-slice j (1024 of the 4096
flattened spatial positions). k/v are computed for the full sequence on every
core of a batch group (cheap, redundant); each core's projection output slice
is complete, so the host only concatenates slices — no collectives.

SPMD: all cores run the identical program. The host rotates each core's copy
of x along t so the core's own slice sits at columns 0:1024 (GroupNorm stats
and the attention contraction over s are invariant to a consistent
permutation of the contracted axis).

Math layout notes:
- scores are computed transposed, S^T[s, t] = k^T q, so softmax's reduction
  runs over the PSUM partition dim; the denominator comes for free from a
  ones-column appended to v^T in the a = w v matmul (output row 64).
- no max-subtraction in softmax: scores are ~N(0, 1) with |s| < ~8, exp is
  safe in fp32 (verified against the reference on host).
- exp is split across engines: heads 2p (hh=0) use the ACT spline exp;
  heads 2p+1 (hh=1) use a Schraudolph fast-exp on the DVE — w_bits_i16 =
  round(s * A + B) bitcast to f16 (max ~3% weight error, ~2e-4 on the
  final output after softmax normalization and head mixing).
- q/k biases are added during the PSUM->SBUF copies; the v bias is folded
  into the projection bias on the host; the attention scale 1/sqrt(ch) is
  folded into exp's scale argument.
- matmuls run in float32r (~1.2e-4 relative rounding, full PE speed).

Performance notes (measured on HW, paired A/B):
- The kernel is power/clock limited: sustained full-rate PE across all 8
  cores trips a clock clamp (K=4/8, half PE clock) ~140us in. The zeros
  "dummy" matmuls into the open av accumulation are load-bearing: they keep
  the HAM activity window busy at low switching power, and the clamp then
  releases in ~17us instead of persisting ~140us (paired cost of removing
  them: +50us). 2x512 cols per sparse j is the measured optimum (1x: +17us,
  3x: +3us).
- GroupNorm rstd is computed on the DVE (bit-hack seed + 2 Newton steps) so
  ACT loads its exp table exactly once, at t~0 - an ACT_TABLE_LOAD swap
  mid-kernel costs 2.7us on the critical path.
- The k-bias PSUM->SBUF copy runs on ACT but is emitted 2 side-slots after
  its matmuls: ACT's 8-deep FIFO otherwise head-of-line-blocks the next exp
  behind an unfinished k matmul.
- Per j, both heads' S matmuls+exps are emitted before both AV accumulations
  so one exp engine's stall cannot cascade into the other head's chain
  through the PE FIFO.
- x DMA: slices first on all three queues (tiny-const DMAs ahead of x cost
  1-2us completion latency each); PE warm-up matmuls are paced by the
  landing x slices (f16 bitcast views) to bridge HAM until the stream.
- Keeping k/v production interleaved with pair-0 attention beats a dedicated
  production phase by ~17us: production rides in the exp-chain stall shadow.
- 512-wide exp halves (to shorten the exp->S bank chain) lose ~22us: the
  extra per-op overhead on ACT/DVE outweighs the chain relief.
"""
import math
from contextlib import ExitStack

import numpy as np

import concourse.bacc as bacc
import concourse.tile as tile
from concourse import mybir
from concourse.bass_utils import run_bass_kernel_spmd

f32 = mybir.dt.float32
f32r = mybir.dt.float32r
f16 = mybir.dt.float16
i16 = mybir.dt.int16
i32 = mybir.dt.int32
FT = mybir.ActivationFunctionType
ALU = mybir.AluOpType

B, C, HH, WW = 2, 256, 64, 64
T = HH * WW           # 4096
TS = T // 4           # 1024 t-columns per core
HEADS = 4
CH = C // HEADS       # 64 channels per head
NT = TS // 512        # 512-wide matmul output tiles per t-slice
SJ = T // 128         # 32 s-tiles
EPS = 1e-5
N_CORES = 8
EXP_SCALE = 1.0 / math.sqrt(CH)  # (1/ch^0.25)^2 folded into exp
# Schraudolph fast exp on the DVE: f16 bits = round(s * SCH_A + SCH_B),
# bitcast int16 -> f16. Shift 45 minimizes max relative error (~3.0%).
SCH_A = EXP_SCALE * (1 << 10) / math.log(2.0)
SCH_B = float(15 * (1 << 10) - 45)

_CACHE: dict = {}


def _build():
    nc = bacc.Bacc("TRN2", target_bir_lowering=False, debug=False,
                   num_devices=N_CORES)

    def dram_in(name, shape, dtype=f32):
        return nc.dram_tensor(name, shape, dtype, kind="ExternalInput").ap()

    x = dram_in("x", [C, T])
    qwt = dram_in("qwt", [C, C], f16)
    kwt = dram_in("kwt", [C, C], f16)
    vwt = dram_in("vwt", [C, C], f16)
    pwt = dram_in("pwt", [C, C], f16)
    qb2 = dram_in("qb2", [128, 2])
    kb2 = dram_in("kb2", [128, 2])
    pb2 = dram_in("pb2", [128, 2])
    nw2 = dram_in("nw2", [128, 2])
    nb2 = dram_in("nb2", [128, 2])
    gsel = dram_in("gsel", [128, 16], f32r)
    gselt = dram_in("gselt", [16, 128], f32r)
    ones = dram_in("ones", [128, 128], f16)
    out = nc.dram_tensor("out", [C, TS], f32, kind="ExternalOutput").ap()

    x2 = x.rearrange("(i p) t -> p i t", i=2)  # [128, 2, 4096] view

    with tile.TileContext(nc) as tc, ExitStack() as ctx:
        sb1 = ctx.enter_context(tc.tile_pool(name="sb1", bufs=1))
        wp = ctx.enter_context(tc.tile_pool(name="wp", bufs=4))
        st = ctx.enter_context(tc.tile_pool(name="st", bufs=2))
        rp = ctx.enter_context(tc.tile_pool(name="rp", bufs=2))
        ps = ctx.enter_context(tc.tile_pool(name="ps", bufs=1, space="PSUM"))
        psa = ctx.enter_context(tc.tile_pool(name="psa", bufs=1, space="PSUM"))

        # ---- persistent tiles ----
        # DMA queue plan (3 queues: SP + ACT hardware DGE ~150 GB/s each,
        # gpsimd software DGE ~86 GB/s). Tiny constants go first, then the
        # 8 x-slices balanced 3/3/2, large weights after x on the queues.
        x_sb = sb1.tile([128, 2, T], f32)
        qb_sb = sb1.tile([128, 2], f32)
        kb_sb = sb1.tile([128, 2], f32)
        pb_sb = sb1.tile([128, 2], f32)
        nw_sb = sb1.tile([128, 2], f32)
        nb_sb = sb1.tile([128, 2], f32)
        gsel_sb = sb1.tile([128, 16], f32r)
        gselt_sb = sb1.tile([16, 128], f32r)
        ones_sb = sb1.tile([128, 128], f16)
        qwt_sb = sb1.tile([128, 2, C], f16)
        kwt_sb = sb1.tile([128, 2, C], f16)
        vwt_sb = sb1.tile([128, 2, C], f16)
        pwt_sb = sb1.tile([128, 2, C], f16)
        eps_sb = sb1.tile([128, 1], f32)
        nc.vector.memset(eps_sb[:], EPS)

        def xdma(eng, i, c2):
            eng.dma_start(out=x_sb[:, i, c2 * 1024:(c2 + 1) * 1024],
                          in_=x2[:, i, c2 * 1024:(c2 + 1) * 1024])

        def wdma(eng, dst, src):
            eng.dma_start(out=dst[:], in_=src.rearrange("(i p) o -> p i o", i=2))

        # x slices first on every queue (tiny-const DMAs ahead of x cost
        # ~1-2us completion latency each); constants and weights after.
        # sync queue: 3 x-slices, group-norm constants, k/v weights
        xdma(nc.sync, 0, 0)
        xdma(nc.sync, 0, 1)
        xdma(nc.sync, 0, 2)
        for dst, src in ((gsel_sb, gsel), (gselt_sb, gselt), (nw_sb, nw2),
                         (nb_sb, nb2), (qb_sb, qb2), (kb_sb, kb2)):
            nc.sync.dma_start(out=dst[:], in_=src[:])
        wdma(nc.sync, kwt_sb, kwt)
        wdma(nc.sync, vwt_sb, vwt)
        # scalar queue: 3 x-slices, then q weights + tail bias
        xdma(nc.scalar, 0, 3)
        xdma(nc.scalar, 1, 0)
        xdma(nc.scalar, 1, 1)
        wdma(nc.scalar, qwt_sb, qwt)
        nc.scalar.dma_start(out=pb_sb[:], in_=pb2[:])
        # gpsimd queue (slow SWDGE): ones (needed mid-head-phase), 2 x-slices
        nc.gpsimd.dma_start(out=ones_sb[:], in_=ones[:])
        xdma(nc.gpsimd, 1, 2)
        xdma(nc.gpsimd, 1, 3)
        wdma(nc.gpsimd, pwt_sb, pwt)

        xn = sb1.tile([128, 2, T], f16)
        k_sb = sb1.tile([128, 2, T], f16)
        q_sb = sb1.tile([128, 2, TS], f16)
        vaug = sb1.tile([128, SJ, HEADS, CH + 1], f16)
        a_sb = sb1.tile([128, 2, TS], f16)

        # Load the natural_log_exp activation table once, at t~0 (GroupNorm
        # rstd uses Ln+Exp and the attention stream uses Exp, so ACT never
        # swaps table sets mid-kernel).
        exp_warm = st.tile([16, 1], f32, name="exp_warm", tag="expw")
        nc.scalar.activation(out=exp_warm[:], in_=eps_sb[0:16, :], func=FT.Exp)

        # PE warm-up: an initial burst on a memset tile ramps HAM to 8/8,
        # then matmuls paced by the landing x slices (f16 bitcast views of
        # the f32 data - values are garbage, the DMA dependency is the
        # point) keep every HAM activity window non-idle until the real
        # matmul stream starts.
        warm16 = sb1.tile([128, 512], f16)
        nc.vector.memset(warm16[:], 1.0)
        warm_ps = ps.tile([128, 512], f32, name="warm_ps", tag="sc0")
        for _ in range(20):
            nc.tensor.matmul(out=warm_ps[:], lhsT=warm16[:, 0:128],
                             rhs=warm16[:], start=True, stop=True)
        warm_order = ((0, 0), (0, 3), (1, 2), (0, 1), (1, 0), (0, 2), (1, 1))
        for n, (i, c2) in enumerate(warm_order):
            reps = 5 if n < 5 else 3
            for r in range(reps):
                base = c2 * 1024 + r * 192
                nc.tensor.matmul(
                    out=warm_ps[:],
                    lhsT=warm16[:, 0:128],
                    rhs=x_sb[:, i, base:base + 256].bitcast(f16),
                    start=True, stop=True,
                )

        # zeros tile for the HAM-keepalive dummies in the attention stream
        zer_sb = sb1.tile([128, 65], f16)
        nc.vector.memset(zer_sb[:], 0.0)

        # ones column of vaug (col CH of every (j, h) slot) - on gpsimd so
        # the wait for the ones DMA doesn't head-of-line-block the DVE
        nc.gpsimd.tensor_copy(
            out=vaug[:, :, :, CH:CH + 1],
            in_=ones_sb[:, 0:SJ * HEADS].rearrange("p (j h) -> p j h", j=SJ),
        )

        # ---- phase A: GroupNorm ----
        stats_all = sb1.tile([128, 2, 8, 6], f32)
        ab = []  # per c-tile (alpha, beta) [128, 2]
        for i in range(2):
            for s in range(8):
                nc.vector.bn_stats(
                    out=stats_all[:, i, s, :],
                    in_=x_sb[:, i, s * 512:(s + 1) * 512],
                )
            hp = tc.high_priority()
            hp.__enter__()
            mv = st.tile([128, 2], f32, name=f"mv_{i}", tag="mv")
            nc.vector.bn_aggr(out=mv[:], in_=stats_all[:, i])
            # me = (mean_c, E[x^2]_c)
            me = st.tile([128, 2], f32, name=f"me_{i}", tag="me")
            nc.vector.tensor_copy(out=me[:, 0:1], in_=mv[:, 0:1])
            nc.vector.tensor_tensor(out=me[:, 1:2], in0=mv[:, 0:1], in1=mv[:, 0:1], op=ALU.mult)
            nc.vector.tensor_add(out=me[:, 1:2], in0=me[:, 1:2], in1=mv[:, 1:2])
            me_r = st.tile([128, 2], f32r, name=f"me_r_{i}", tag="me_r")
            nc.vector.tensor_copy(out=me_r[:], in_=me[:])
            # group sums: [16, 2] = sum over the 8 channels of each group
            gs_ps = ps.tile([16, 2], f32, name=f"gs_ps_{i}", tag="sc0")
            nc.tensor.matmul(out=gs_ps[:], lhsT=gsel_sb[:], rhs=me_r[:], start=True, stop=True)
            gstats = st.tile([16, 2], f32, name=f"gstats_{i}", tag="gstats")
            nc.vector.tensor_scalar_mul(out=gstats[:], in0=gs_ps[:], scalar1=1.0 / 8.0)
            tmp1 = st.tile([16, 1], f32, name=f"tmp1_{i}", tag="tmp1")
            nc.vector.tensor_tensor(out=tmp1[:], in0=gstats[:, 0:1], in1=gstats[:, 0:1], op=ALU.mult)
            nc.vector.tensor_sub(out=gstats[:, 1:2], in0=gstats[:, 1:2], in1=tmp1[:])
            # rstd = 1/sqrt(var + eps) entirely on the DVE (bit-hack seed +
            # two Newton steps) so ACT keeps its exp table resident for the
            # whole kernel - no ACT_TABLE_LOAD swaps.
            v_t = st.tile([16, 1], f32, name=f"v_{i}", tag="rsq_v")
            nc.vector.tensor_scalar_add(out=v_t[:], in0=gstats[:, 1:2],
                                        scalar1=eps_sb[0:16, :])
            hsh = st.tile([16, 1], f32, name=f"h_{i}", tag="rsq_h")
            nc.vector.tensor_scalar_mul(out=hsh[:], in0=v_t[:], scalar1=0.5)
            # seed bits = 0x5F3759DF - bits(v)/2, computed in fp32 (the int32
            # input converts to fp32 in the ALU; <=0.5-bit error is nothing
            # against the 3.4% seed error), then rounded back to int32
            u_t = st.tile([16, 1], f32, name=f"u_{i}", tag="rsq_u")
            nc.vector.tensor_scalar(out=u_t[:], in0=v_t[:].bitcast(i32),
                                    scalar1=-0.5, scalar2=float(0x5F3759DF),
                                    op0=ALU.mult, op1=ALU.add)
            y_t = st.tile([16, 1], f32, name=f"y_{i}", tag="rsq_y")
            nc.vector.tensor_copy(out=y_t[:].bitcast(i32), in_=u_t[:])
            t_t = st.tile([16, 1], f32, name=f"t_{i}", tag="rsq_t")
            for _ in range(2):  # Newton: y *= 1.5 - h*y*y
                nc.vector.tensor_tensor(out=t_t[:], in0=y_t[:], in1=y_t[:], op=ALU.mult)
                nc.vector.tensor_tensor(out=t_t[:], in0=t_t[:], in1=hsh[:], op=ALU.mult)
                nc.vector.tensor_scalar(out=t_t[:], in0=t_t[:], scalar1=-1.0,
                                        scalar2=1.5, op0=ALU.mult, op1=ALU.add)
                nc.vector.tensor_tensor(out=y_t[:], in0=y_t[:], in1=t_t[:], op=ALU.mult)
            nc.vector.tensor_copy(out=gstats[:, 1:2], in_=y_t[:])
            gstats_r = st.tile([16, 2], f32r, name=f"gstats_r_{i}", tag="gstats_r")
            nc.vector.tensor_copy(out=gstats_r[:], in_=gstats[:])
            # broadcast to channels: [128, 2] = (mean_c, rstd_c)
            ch_ps = ps.tile([128, 2], f32, name=f"ch_ps_{i}", tag="sc1")
            nc.tensor.matmul(out=ch_ps[:], lhsT=gselt_sb[:], rhs=gstats_r[:], start=True, stop=True)
            ab_i = st.tile([128, 2], f32, name=f"ab_{i}", tag="ab", bufs=2)
            nc.vector.tensor_tensor(out=ab_i[:, 0:1], in0=ch_ps[:, 1:2], in1=nw_sb[:, i:i + 1], op=ALU.mult)
            tmp2 = st.tile([128, 1], f32, name=f"tmp2_{i}", tag="tmp2")
            nc.vector.tensor_tensor(out=tmp2[:], in0=ch_ps[:, 0:1], in1=ab_i[:, 0:1], op=ALU.mult)
            nc.vector.tensor_sub(out=ab_i[:, 1:2], in0=nb_sb[:, i:i + 1], in1=tmp2[:])
            hp.__exit__(None, None, None)
            ab.append(ab_i)

        # apply affine -> xn (f16) in 1024-col chunks, alternating DVE/ACT to
        # halve the post-stats latency
        for i in range(2):
            for c4 in range(4):
                sl = slice(c4 * 1024, (c4 + 1) * 1024)
                if c4 % 2 == 0:
                    nc.vector.tensor_scalar(
                        out=xn[:, i, sl], in0=x_sb[:, i, sl],
                        scalar1=ab[i][:, 0:1], scalar2=ab[i][:, 1:2],
                        op0=ALU.mult, op1=ALU.add,
                    )
                else:
                    nc.scalar.activation(
                        out=xn[:, i, sl], in_=x_sb[:, i, sl],
                        func=FT.Identity,
                        scale=ab[i][:, 0:1], bias=ab[i][:, 1:2],
                    )

        # ---- phase B: qkv projections ----
        # last HAM-keepalive matmuls, paced on the last x slice
        for r in range(3):
            nc.tensor.matmul(
                out=warm_ps[:],
                lhsT=warm16[:, 0:128],
                rhs=x_sb[:, 1, 3 * 1024 + r * 192:3 * 1024 + r * 192 + 256].bitcast(f16),
                start=True, stop=True,
            )
        # q: [128, 2(pair), 1024]
        for p in range(2):
            q_ps = ps.tile([128, TS], f32, name=f"q_ps_{p}", tag=f"sc{p}")
            for nt in range(NT):
                for i in range(2):
                    nc.tensor.matmul(
                        out=q_ps[:, nt * 512:(nt + 1) * 512],
                        lhsT=qwt_sb[:, i, p * 128:(p + 1) * 128],
                        rhs=xn[:, i, nt * 512:(nt + 1) * 512],
                        start=(i == 0), stop=(i == 1),
                    )
            nc.vector.tensor_scalar_add(out=q_sb[:, p, :], in0=q_ps[:], scalar1=qb_sb[:, p:p + 1])
        # k and v^T production interleaved with attention consumption:
        # after chunk c4's k/v^T are emitted, attention js of chunk c4-1 for
        # pair 0 run, keeping ACT (exp) continuously busy from ~40us on.
        def k_thunks(c4):
            """Per pair: nt=0 matmuls, nt=1 matmuls; the PSUM->SBUF bias adds
            trail both pairs so the ACT op never head-of-line-blocks an exp
            behind an unfinished k matmul."""
            units = []
            cells = [{}, {}]
            for p in range(2):
                cell = cells[p]
                def mk_k0(p=p, cell=cell):
                    cell["t"] = ps.tile([128, 1024], f32, name=f"k_ps_{p}_{c4}", tag=f"sc{p}")
                    for i in range(2):
                        nc.tensor.matmul(
                            out=cell["t"][:, 0:512],
                            lhsT=kwt_sb[:, i, p * 128:(p + 1) * 128],
                            rhs=xn[:, i, c4 * 1024: c4 * 1024 + 512],
                            start=(i == 0), stop=(i == 1),
                        )
                def mk_k1(p=p, cell=cell):
                    for i in range(2):
                        nc.tensor.matmul(
                            out=cell["t"][:, 512:1024],
                            lhsT=kwt_sb[:, i, p * 128:(p + 1) * 128],
                            rhs=xn[:, i, c4 * 1024 + 512: c4 * 1024 + 1024],
                            start=(i == 0), stop=(i == 1),
                        )
                units += [mk_k0, mk_k1]
            for p in range(2):
                def mk_kb(p=p, cell=cells[p]):
                    # bias add on ACT (reads PSUM at full rate; keeps the DVE
                    # free for the Schraudolph exp of the hh=1 heads)
                    nc.scalar.activation(
                        out=k_sb[:, p, c4 * 1024:(c4 + 1) * 1024], in_=cell["t"][:],
                        func=FT.Identity, bias=kb_sb[:, p:p + 1],
                    )
                units.append(mk_kb)
            return units

        def v_thunk(j):
            def mk_v(j=j):
                vt_ps = ps.tile([128, C], f32, name=f"vt_ps_{j}", tag=f"sc{j % 2}")
                for i in range(2):
                    nc.tensor.matmul(
                        out=vt_ps[:], lhsT=xn[:, i, j * 128:(j + 1) * 128],
                        rhs=vwt_sb[:, i, :], start=(i == 0), stop=(i == 1),
                    )
                nc.vector.tensor_copy(
                    out=vaug[:, j, :, 0:CH],
                    in_=vt_ps.rearrange("p (h c) -> p h c", h=HEADS),
                )
            return mk_v

        att = {}  # per-pair attention state: (avs, prev_w)
        att = {}  # per-pair attention state: (avs, prev_w)

        def emit_att(p, js, side=None):
            avs, prev_w = att[p]
            side = list(side or [])
            si = 0
            per_j = max(1, (len(side) + len(js) - 1) // len(js)) if side else 0
            for j in js:
                cur_w = [None, None]
                # both heads' score matmuls + exps first (so a stall in one
                # head's exp chain can't block the other via the PE FIFO),
                # then both heads' AV accumulations (their w is ready).
                for hh in range(2):
                    h = 2 * p + hh
                    off = hh * CH
                    s_ps = ps.tile([128, TS], f32, name=f"s_ps_{h}_{j}", tag=f"sc{hh}")
                    for nt in range(NT):
                        nc.tensor.matmul(
                            out=s_ps[:, nt * 512:(nt + 1) * 512],
                            lhsT=k_sb[off:off + CH, p, j * 128:(j + 1) * 128],
                            rhs=q_sb[off:off + CH, p, nt * 512:(nt + 1) * 512],
                            start=True, stop=True,
                        )
                    w_t = wp.tile([128, TS], f16, name=f"w_{h}_{j}", tag="w")
                    cur_w[hh] = w_t
                    if hh == 0:
                        nc.scalar.activation(out=w_t[:], in_=s_ps[:], func=FT.Exp,
                                             scale=EXP_SCALE)
                    else:
                        nc.vector.tensor_scalar(
                            out=w_t[:].bitcast(i16), in0=s_ps[:],
                            scalar1=SCH_A, scalar2=SCH_B,
                            op0=ALU.mult, op1=ALU.add,
                        )
                for hh in range(2):
                    h = 2 * p + hh
                    if prev_w[hh] is not None:
                        for nt in range(NT):
                            nc.tensor.matmul(
                                out=avs[hh][:, nt * 512:(nt + 1) * 512],
                                lhsT=vaug[:, j - 1, h, :],
                                rhs=prev_w[hh][:, nt * 512:(nt + 1) * 512],
                                start=(j - 1 == 0), stop=False,
                            )
                if side and si < len(side):
                    for t in side[si:si + per_j]:
                        t()
                    si += per_j
                elif 2 <= j < SJ - 1:
                    # HAM-keepalive matmuls (add zeros to the open av
                    # accumulation) so no activity window goes idle
                    for f in range(2):
                        nc.tensor.matmul(
                            out=avs[0][:, (f % 2) * 512:(f % 2 + 1) * 512],
                            lhsT=zer_sb[:], rhs=xn[:, 0, 0:512],
                            start=False, stop=False, skip_group_check=True,
                        )
                prev_w = cur_w
            for t in side[si:]:
                t()
            att[p] = (avs, prev_w)

        def finish_stop(p):
            """Final av accumulation + evacuate av banks (raw copies)."""
            avs, prev_w = att[p]
            raws = []
            for hh in range(2):
                h = 2 * p + hh
                for nt in range(NT):
                    nc.tensor.matmul(
                        out=avs[hh][:, nt * 512:(nt + 1) * 512],
                        lhsT=vaug[:, SJ - 1, h, :],
                        rhs=prev_w[hh][:, nt * 512:(nt + 1) * 512],
                        start=False, stop=True,
                    )
            for hh in range(2):
                h = 2 * p + hh
                av = avs[hh]
                araw = rp.tile([CH, TS], f32, name=f"araw_{h}", tag=f"araw{hh}")
                d16 = rp.tile([1, TS], f16, name=f"d16_{h}", tag="d16")
                # split the four PSUM evacuations across ACT and DVE so the
                # two heads drain in parallel instead of serializing on DVE
                if hh == 0:
                    nc.scalar.activation(out=araw[:], in_=av[0:CH, :], func=FT.Copy)
                    nc.vector.tensor_copy(out=d16[:], in_=av[CH:CH + 1, :])
                else:
                    nc.vector.tensor_copy(out=araw[:], in_=av[0:CH, :])
                    nc.scalar.activation(out=d16[:], in_=av[CH:CH + 1, :], func=FT.Copy)
                raws.append((araw, d16))
            return raws

        def normalize_thunks(p, raws):
            """Per-head normalize, emitted as side work inside the next pair."""
            thunks = []
            for hh in range(2):
                h = 2 * p + hh
                off = hh * CH
                araw, d16 = raws[hh]
                def mk(h=h, off=off, araw=araw, d16=d16, hh=hh, p=p):
                    drep_ps = ps.tile([CH, TS], f32, name=f"drep_ps_{h}", tag=f"sc{hh}")
                    for nt in range(NT):
                        nc.tensor.matmul(
                            out=drep_ps[:, nt * 512:(nt + 1) * 512],
                            lhsT=ones_sb[0:1, 0:CH],
                            rhs=d16[:, nt * 512:(nt + 1) * 512],
                            start=True, stop=True,
                        )
                    rrep_sb = rp.tile([CH, TS], f32, name=f"rrep_sb_{h}", tag="rrep")
                    nc.vector.reciprocal_approx_fast(out=rrep_sb[:], in_=drep_ps[:])
                    nc.vector.tensor_tensor(
                        out=a_sb[off:off + CH, p, :], in0=araw[:], in1=rrep_sb[:],
                        op=ALU.mult,
                    )
                thunks.append(mk)
            return thunks

        att[0] = ([psa.tile([CH + 1, TS], f32, name=f"av_{hh}", tag=f"acc{hh}")
                   for hh in range(2)], [None, None])
        for t in k_thunks(0):
            t()
        for j in range(4):
            v_thunk(j)()
        for c4 in range(4):
            side = []
            if c4 < 3:
                side += k_thunks(c4 + 1)
            side += [v_thunk(j) for j in range(8 * c4 + 4, min(8 * c4 + 12, SJ))]
            emit_att(0, range(8 * c4, 8 * c4 + 8), side=side)
        raws0 = finish_stop(0)
        att[1] = ([psa.tile([CH + 1, TS], f32, name=f"av_{2 + hh}", tag=f"acc{hh}")
                   for hh in range(2)], [None, None])
        emit_att(1, range(SJ), side=normalize_thunks(0, raws0))
        raws1 = finish_stop(1)
        for t in normalize_thunks(1, raws1):
            t()

        # ---- phase D: projection + residual ----
        for m in range(2):
            h_ps = ps.tile([128, TS], f32, name=f"h_ps_{m}", tag=f"sc{m}")
            for nt in range(NT):
                for i in range(2):
                    nc.tensor.matmul(
                        out=h_ps[:, nt * 512:(nt + 1) * 512],
                        lhsT=pwt_sb[:, i, m * 128:(m + 1) * 128],
                        rhs=a_sb[:, i, nt * 512:(nt + 1) * 512],
                        start=(i == 0), stop=(i == 1),
                    )
            o_sb = wp.tile([128, TS], f32, name=f"o_sb_{m}", tag="w")
            # bias+residual and the output DMA in 512-col halves on separate
            # queues: the first half's DMA overlaps the second half's compute
            # and the final DMA is half as long after the last compute op
            engs = ((nc.sync, nc.scalar), (nc.gpsimd, nc.sync))[m]
            for half in range(2):
                sl = slice(half * 512, (half + 1) * 512)
                nc.vector.scalar_tensor_tensor(
                    out=o_sb[:, sl], in0=h_ps[:, sl], scalar=pb_sb[:, m:m + 1],
                    in1=x_sb[:, m, sl], op0=ALU.add, op1=ALU.add,
                )
                engs[half].dma_start(
                    out=out[m * 128:(m + 1) * 128, sl], in_=o_sb[:, sl])

    nc.compile()
    return nc


def _host_inputs(x, norm_w, norm_b, qkv_w, qkv_b, proj_w, proj_b):
    """Build the 8 per-core input maps (all float32 numpy)."""
    x = np.ascontiguousarray(np.asarray(x, dtype=np.float32)).reshape(B, C, T)
    norm_w = np.asarray(norm_w, dtype=np.float32)
    norm_b = np.asarray(norm_b, dtype=np.float32)
    qkv_w = np.asarray(qkv_w, dtype=np.float32)
    qkv_b = np.asarray(qkv_b, dtype=np.float32)
    proj_w = np.asarray(proj_w, dtype=np.float32)
    proj_b = np.asarray(proj_b, dtype=np.float32)

    # head-major row gathers of the qkv conv
    q_rows = np.concatenate([np.arange(192 * h, 192 * h + 64) for h in range(HEADS)])
    k_rows = q_rows + 64
    v_rows = q_rows + 128
    qwt = np.ascontiguousarray(qkv_w[q_rows].T.astype(np.float16))
    kwt = np.ascontiguousarray(qkv_w[k_rows].T.astype(np.float16))
    vwt = np.ascontiguousarray(qkv_w[v_rows].T.astype(np.float16))
    pwt = np.ascontiguousarray(proj_w.T.astype(np.float16))

    def as2(v):  # (256,) -> [128, 2] with column p = channels 128p..128p+128
        return np.ascontiguousarray(v.reshape(2, 128).T)

    qb2 = as2(qkv_b[q_rows])
    kb2 = as2(qkv_b[k_rows])
    # v bias folded into projection bias (a_norm lacks +vb; h += proj_w @ vb)
    vb_nat = qkv_b[v_rows]  # natural channel order == head-major for v
    pb2 = as2(proj_b + proj_w @ vb_nat)
    nw2 = as2(norm_w)
    nb2 = as2(norm_b)

    gsel = np.zeros((128, 16), np.float32)
    gsel[np.arange(128), np.arange(128) // 8] = 1.0
    gselt = np.ascontiguousarray(gsel.T)
    ones = np.ones((128, 128), np.float16)

    shared = dict(qwt=qwt, kwt=kwt, vwt=vwt, pwt=pwt, qb2=qb2, kb2=kb2,
                  pb2=pb2, nw2=nw2, nb2=nb2, gsel=gsel, gselt=gselt, ones=ones)
    in_maps = []
    for core in range(N_CORES):
        b, j = core // 4, core % 4
        xr = np.concatenate([x[b][:, j * TS:], x[b][:, :j * TS]], axis=1)
        in_maps.append({"x": np.ascontiguousarray(xr), **shared})
    return in_maps


def _run(in_maps, **kw):
    if "nc" not in _CACHE:
        _CACHE["nc"] = _build()
    return run_bass_kernel_spmd(_CACHE["nc"], in_maps, list(range(N_CORES)), **kw)


def kernel(x, norm_w, norm_b, qkv_w, qkv_b, proj_w, proj_b):
    in_maps = _host_inputs(x, norm_w, norm_b, qkv_w, qkv_b, proj_w, proj_b)
    res = _run(in_maps)
    out = np.empty((B, C, T), np.float32)
    for core in range(N_CORES):
        b, j = core // 4, core % 4
        out[b][:, j * TS:(j + 1) * TS] = res.results[core]["out"]
    return out.reshape(B, C, HH, WW)

